# revision 2
# baseline (speedup 1.0000x reference)
"""Trainium2 Bass kernel for nn_Compressor (NSA-style windowed KV compression).

Math (per reference):
  kv   = x @ wkv_w.T                     [B, S, 1024]
  gate = sigmoid(x @ wgate_w.T)
  kv   = kv * gate + tile(ape)           (ape per position-within-window)
  kv   = mean over windows of 4          [B, S/4, 2, 512]
  out  = norm_w * kv * rsqrt(mean(kv^2, -1) + eps)   [B, S/2, 512]

Distribution: x flattened to [B*S, 4096] = [16384, 4096], sharded into 8
contiguous 2048-row blocks (whole windows per shard); weights replicated.
Each core computes its [1024, 512] output shard; host concatenates and
applies the norm_w elementwise multiply (exact, off the device).

Per-core strategy:
  - kv matmul in fp16 (10-bit mantissa; values are O(1) so fp16 range is
    safe), pre-transposed so the contraction dim (D) is the SBUF partition
    dim; fp32 accumulation in PSUM. 32 matmuls of 512 free per (128-row,
    512-col) group.
  - gate matmul in fp8e4m3 with DoubleRow perf mode: each instruction
    contracts TWO 128-chunks of D (measured: same 512-cycle stream as one
    fp16 matmul, i.e. 2x FLOP rate), so the gate costs 16 instructions per
    group instead of 32. Sigmoid attenuates the fp8 input error; wgate is
    pre-scaled x64 so its values clear the e4m3 subnormal floor, undone by
    the sigmoid's scale operand. Adds ~1.4e-2 output rel err (vs the 2e-2
    correctness budget); kv stays fp16 because its error hits the output
    un-attenuated (fp8 kv would be ~3.5e-2).
  - epilogue per group: sigmoid (ACT), kvg = psum*gate (DVE),
    +ape fused with the f16 downcast (DVE), window-pool via a PE matmul
    against a [128, 32] 0.25-indicator matrix, pooled copy (ACT) +
    sum-of-squares via ACT Square with the accumulator output. All ACT
    functions here live in the same activation table (sigmoid set).
  - RMSNorm finals (sqrt -> reciprocal -> scale) are deferred and batched
    16 groups at a time so the ACT table switch between the sigmoid and
    sqrt sets happens 4x per pass instead of 64x.
  - weights resident in SBUF (12.6 MiB); x streamed once per pass with
    double buffering; block-0 fp8 x arrives in dc-sliced pieces interleaved
    with the weight stream so the first matmuls start early.
"""

import sys

sys.path.insert(0, "/opt/trn_rl_repo")

import numpy as np
import ml_dtypes

import concourse.tile as tile
from concourse import bacc, mybir
from concourse.bass_utils import run_bass_kernel_spmd

HALF = np.float16
F8 = ml_dtypes.float8_e4m3

N_CORES = 8
B, S, D = 4, 4096, 4096
R = 4                  # compress ratio (window)
HD = 512               # head dim
OD = 1024              # coff * head_dim
EPS = 1e-6

ROWS = (B * S) // N_CORES      # 2048 sequence rows per core
DC = D // 128                  # 32 contraction chunks
NP = DC // 2                   # 16 DoubleRow chunk pairs
NT = ROWS // 128               # 16 s-tiles per core
SBLK = 256                     # x columns loaded per DMA block (2 s-tiles)
NW_TILE = 128 // R             # 32 windows per s-tile
WSCALE = 64.0                  # wgate pre-scale (clears e4m3 subnormals)

_CACHED_NC = None


def _build_nc(reps=1, do_compile=True):
    nc = bacc.Bacc("TRN2", target_bir_lowering=False, debug=False,
                   num_devices=N_CORES)
    f32 = mybir.dt.float32
    f16 = mybir.dt.float16
    f8 = mybir.dt.float8e4
    DRM = mybir.MatmulPerfMode.DoubleRow

    xh8 = nc.dram_tensor("xh8", [D, ROWS], f8, kind="ExternalInput").ap()
    x16 = nc.dram_tensor("x16", [D, ROWS], f16, kind="ExternalInput").ap()
    wkvt = nc.dram_tensor("wkvt", [D, OD], f16, kind="ExternalInput").ap()
    wg8 = nc.dram_tensor("wg8", [D, OD], f8, kind="ExternalInput").ap()
    apeb = nc.dram_tensor("apeb", [128, OD], f32, kind="ExternalInput").ap()
    poolm = nc.dram_tensor("poolm", [128, NW_TILE], f16, kind="ExternalInput").ap()
    out = nc.dram_tensor("out", [ROWS // R * 2, HD], f32, kind="ExternalOutput").ap()

    # [p, dc, n] views with the contraction dim on partitions
    xh8_v = xh8.rearrange("(dc p) s -> p dc s", p=128)
    x16_v = x16.rearrange("(dc p) s -> p dc s", p=128)
    wkvt_v = wkvt.rearrange("(dc p) o -> p dc o", p=128)
    wg8_v = wg8.rearrange("(dc p) o -> p dc o", p=128)
    out_v = out.rearrange("(w two) h -> w two h", two=2)

    with tile.TileContext(nc) as tc:
        with (
            tc.tile_pool(name="const", bufs=1) as const_pool,
            tc.tile_pool(name="wpool", bufs=1) as wpool,
            tc.tile_pool(name="xpool", bufs=2) as xpool,
            tc.tile_pool(name="acts", bufs=2) as acts,
            tc.tile_pool(name="small", bufs=2) as small,
            tc.tile_pool(name="pp", bufs=NT) as p16_pool,
            tc.tile_pool(name="sq", bufs=NT * 2) as ssq_pool,
            tc.tile_pool(name="fin", bufs=4) as fin_pool,
            tc.tile_pool(name="mm", bufs=3, space="PSUM") as psum_pool,
            tc.tile_pool(name="pl", bufs=2, space="PSUM") as pool_psum,
        ):
            # Weight slices, one dc-pair per tile so a [:, :, cslice] view is
            # directly a DoubleRow rhs ([128, 2, 512]).
            wkv_sl, wg_sl = [], []
            xb0 = []
            XSL0 = 8  # dc chunks per block-0 x DMA slice

            for d in range(NP):
                if d % (NP // (DC // XSL0)) == 0:
                    s0 = d // (NP // (DC // XSL0))
                    t = const_pool.tile([128, XSL0, SBLK], f8,
                                        tag=f"x0_{s0}", name=f"x0_{s0}")
                    nc.sync.dma_start(
                        t[:], xh8_v[:, s0 * XSL0:(s0 + 1) * XSL0, 0:SBLK])
                    xb0.append(t)
                t = wpool.tile([128, 2, OD], f16, tag=f"wkv{d}")
                nc.sync.dma_start(t[:], wkvt_v[:, 2 * d:2 * d + 2, :])
                wkv_sl.append(t)
                t = wpool.tile([128, 2, OD], f8, tag=f"wg{d}")
                nc.sync.dma_start(t[:], wg8_v[:, 2 * d:2 * d + 2, :])
                wg_sl.append(t)

            apeb_sb = const_pool.tile([128, OD], f32)
            nc.sync.dma_start(apeb_sb[:], apeb)
            poolm_sb = const_pool.tile([128, NW_TILE], f16)
            nc.sync.dma_start(poolm_sb[:], poolm)
            eps_sb = const_pool.tile([128, 1], f32)
            nc.gpsimd.memset(eps_sb[:], EPS)

            def load_xblk(key, view, dtype, blk):
                t = xpool.tile([128, DC, SBLK], dtype, tag=f"xt_{key}")
                nc.sync.dma_start(t[:], view[:, :, blk * SBLK:(blk + 1) * SBLK])
                return t

            def xop(xt, d, j):
                # lhsT [128, 2, 128] for DoubleRow: dc-pair d, s-tile j
                if isinstance(xt, list):  # block-0 dc-sliced tiles
                    t = xt[(2 * d) // XSL0]
                    return t[:, (2 * d) % XSL0:(2 * d) % XSL0 + 2,
                             j * 128:(j + 1) * 128]
                return xt[:, 2 * d:2 * d + 2, j * 128:(j + 1) * 128]

            def epilogue_mid(ps_kv, ps_g, i, c):
                # Per-group epilogue through the pooled sum-of-squares; the
                # sqrt-based finals are batched (epilogue_fin) so the ACT
                # table switches (sigmoid<->sqrt sets) are amortized.
                gate_sb = acts.tile([128, HD], f32, tag="gate")
                nc.scalar.activation(gate_sb[:], ps_g[:],
                                     mybir.ActivationFunctionType.Sigmoid,
                                     scale=1.0 / WSCALE)
                kvg_sb = acts.tile([128, HD], f32, tag="kvg")
                nc.vector.tensor_mul(kvg_sb[:], ps_kv[:], gate_sb[:])
                kvg16 = acts.tile([128, HD], f16, tag="kvg16")
                nc.vector.tensor_add(kvg16[:], kvg_sb[:],
                                     apeb_sb[:, c * HD:(c + 1) * HD])
                pooled_ps = pool_psum.tile([NW_TILE, HD], f32, tag="pooled")
                nc.tensor.matmul(pooled_ps[:], poolm_sb[:], kvg16[:],
                                 start=True, stop=True)
                p16 = p16_pool.tile([NW_TILE, HD], f16, tag="p16")
                nc.scalar.activation(p16[:], pooled_ps[:],
                                     mybir.ActivationFunctionType.Copy)
                sq = small.tile([NW_TILE, HD], f32, tag="sqj")
                ssq = ssq_pool.tile([NW_TILE, 1], f32, tag="ssq")
                nc.scalar.activation(sq[:], pooled_ps[:],
                                     mybir.ActivationFunctionType.Square,
                                     accum_out=ssq[:])
                return (p16, ssq, i, c)

            def epilogue_fin(p16, ssq, i, c):
                std = small.tile([NW_TILE, 1], f32, tag="std")
                nc.scalar.activation(std[:], ssq[:],
                                     mybir.ActivationFunctionType.Sqrt,
                                     bias=eps_sb[:NW_TILE, :], scale=1.0 / HD)
                rinv = small.tile([NW_TILE, 1], f32, tag="rinv")
                nc.vector.reciprocal(rinv[:], std[:])
                onorm = fin_pool.tile([NW_TILE, HD], f32, tag="onorm")
                nc.scalar.mul(onorm[:], p16[:], rinv[:])
                nc.sync.dma_start(
                    out_v[i * NW_TILE:(i + 1) * NW_TILE, c, :], onorm[:])

            pending = []
            fins = []

            def flush(keep):
                while len(pending) > keep:
                    fins.append(epilogue_mid(*pending.pop(0)))

            def drain_fins():
                for f in fins:
                    epilogue_fin(*f)
                fins.clear()

            for _rep in range(reps):
                for blk in range(NT * 128 // SBLK):
                    first = _rep == 0 and blk == 0
                    kv_t = load_xblk("x16", x16_v, f16, blk)
                    xh_t = xb0 if first else load_xblk("xh", xh8_v, f8, blk)
                    for j in range(SBLK // 128):
                        for c in range(2):
                            i = blk * (SBLK // 128) + j
                            csl = slice(c * HD, (c + 1) * HD)
                            ps_kv = psum_pool.tile([128, HD], f32, tag="ps_kv")
                            ps_g = psum_pool.tile([128, HD], f32, tag="ps_g")
                            for dc in range(DC):
                                nc.tensor.matmul(
                                    ps_kv[:],
                                    kv_t[:, dc, j * 128:(j + 1) * 128],
                                    wkv_sl[dc // 2][:, dc % 2, csl],
                                    start=(dc == 0), stop=(dc == DC - 1))
                            for d in range(NP):
                                nc.tensor.matmul(
                                    ps_g[:], xop(xh_t, d, j),
                                    wg_sl[d][:, :, csl],
                                    start=(d == 0), stop=(d == NP - 1),
                                    perf_mode=DRM)
                            pending.append((ps_kv, ps_g, i, c))
                            flush(1)
                            if len(fins) >= NT:
                                drain_fins()
                flush(0)
                drain_fins()

    if do_compile:
        nc.compile()
    return nc


def _get_nc():
    global _CACHED_NC
    if _CACHED_NC is None:
        _CACHED_NC = _build_nc()
    return _CACHED_NC


def _prep_in_maps(x, wkv_w, wgate_w, ape, norm_w):
    x = np.asarray(x, dtype=np.float32)
    wkv_w = np.asarray(wkv_w, dtype=np.float32)
    wgate_w = np.asarray(wgate_w, dtype=np.float32)
    ape = np.asarray(ape, dtype=np.float32)

    xb = x.reshape(B * S, D)
    xh = xb.astype(F8)
    x16 = xb.astype(HALF)

    common = {
        "wkvt": np.ascontiguousarray(wkv_w.astype(HALF).T),          # [D, OD]
        "wg8": np.ascontiguousarray((wgate_w.T * WSCALE).astype(F8)),
        "apeb": np.ascontiguousarray(np.tile(ape, (128 // R, 1))),   # [128, OD]
    }
    poolm = np.zeros((128, NW_TILE), np.float32)
    poolm[np.arange(128), np.arange(128) // R] = 1.0 / R
    common["poolm"] = poolm.astype(HALF)

    in_maps = []
    for k in range(N_CORES):
        rows = slice(k * ROWS, (k + 1) * ROWS)
        m = dict(common)
        m["xh8"] = np.ascontiguousarray(xh[rows, :].T)    # [D, ROWS]
        m["x16"] = np.ascontiguousarray(x16[rows, :].T)
        in_maps.append(m)
    return in_maps


def kernel(x, wkv_w, wgate_w, ape, norm_w):
    nc = _get_nc()
    in_maps = _prep_in_maps(x, wkv_w, wgate_w, ape, norm_w)
    try:
        res = run_bass_kernel_spmd(nc, in_maps, list(range(N_CORES)))
    except Exception:
        # Transient axon-transport failures are retryable; a wedged device
        # (NRT_EXEC_UNIT_UNRECOVERABLE) recovers with a fresh PJRT session.
        try:
            import jax
            jax.clear_backends()
        except Exception:
            pass
        res = run_bass_kernel_spmd(nc, in_maps, list(range(N_CORES)))
    shards = [res.results[k]["out"] for k in range(N_CORES)]
    full = np.concatenate(shards, axis=0).reshape(B, S // R * 2, HD)
    # norm_w multiplies the output elementwise along the head dim; applied
    # on the host, exactly.
    return full * np.asarray(norm_w, dtype=np.float32)[None, None, :]


# revision 3
# speedup vs baseline: 1.5713x; 1.5713x over previous
"""Trainium2 Bass kernel for nn_Compressor (NSA-style windowed KV compression).

Math (per reference):
  kv   = x @ wkv_w.T                     [B, S, 1024]
  gate = sigmoid(x @ wgate_w.T)
  kv   = kv * gate + tile(ape)           (ape per position-within-window)
  kv   = mean over windows of 4          [B, S/4, 2, 512]
  out  = norm_w * kv * rsqrt(mean(kv^2, -1) + eps)   [B, S/2, 512]

Distribution: x flattened to [B*S, 4096] = [16384, 4096], sharded into 8
contiguous 2048-row blocks (whole windows per shard); weights replicated.
Each core computes its [1024, 512] output shard; host concatenates and
applies the norm_w elementwise multiply (exact, off the device).

Per-core strategy:
  - kv matmul in fp16 (10-bit mantissa; values are O(1) so fp16 range is
    safe), pre-transposed so the contraction dim (D) is the SBUF partition
    dim; fp32 accumulation in PSUM. 32 matmuls of 512 free per (128-row,
    512-col) group.
  - gate matmul in fp8e4m3 with DoubleRow perf mode: each instruction
    contracts TWO 128-chunks of D (measured: same 512-cycle stream as one
    fp16 matmul, i.e. 2x FLOP rate), so the gate costs 16 instructions per
    group instead of 32. Sigmoid attenuates the fp8 input error; wgate is
    pre-scaled x64 so its values clear the e4m3 subnormal floor, undone by
    the sigmoid's scale operand. Adds ~1.4e-2 output rel err (vs the 2e-2
    correctness budget); kv stays fp16 because its error hits the output
    un-attenuated (fp8 kv would be ~3.5e-2).
  - epilogue per group: sigmoid (ACT), kvg = psum*gate (DVE),
    +ape fused with the f16 downcast (DVE), window-pool via a PE matmul
    against a [128, 32] 0.25-indicator matrix, pooled copy (ACT) +
    sum-of-squares via ACT Square with the accumulator output. All ACT
    functions here live in the same activation table (sigmoid set).
  - RMSNorm finals (sqrt -> reciprocal -> scale) are deferred and batched
    16 groups at a time so the ACT table switch between the sigmoid and
    sqrt sets happens 4x per pass instead of 64x.
  - weights resident in SBUF (12.6 MiB); x streamed once per pass with
    double buffering; block-0 fp8 x arrives in dc-sliced pieces interleaved
    with the weight stream so the first matmuls start early.
"""

import sys

sys.path.insert(0, "/opt/trn_rl_repo")

import numpy as np
import ml_dtypes

import concourse.tile as tile
from concourse import bacc, mybir
from concourse.bass_utils import run_bass_kernel_spmd

HALF = np.float16
F8 = ml_dtypes.float8_e4m3

N_CORES = 8
B, S, D = 4, 4096, 4096
R = 4                  # compress ratio (window)
HD = 512               # head dim
OD = 1024              # coff * head_dim
EPS = 1e-6

ROWS = (B * S) // N_CORES      # 2048 sequence rows per core
DC = D // 128                  # 32 contraction chunks
NP = DC // 2                   # 16 DoubleRow chunk pairs
NT = ROWS // 128               # 16 s-tiles per core
SBLK = 256                     # x columns loaded per DMA block (2 s-tiles)
NW_TILE = 128 // R             # 32 windows per s-tile
WSCALE = 64.0                  # wgate pre-scale (clears e4m3 subnormals)

_CACHED_NC = None


def _build_nc(reps=1, do_compile=True):
    nc = bacc.Bacc("TRN2", target_bir_lowering=False, debug=False,
                   num_devices=N_CORES)
    f32 = mybir.dt.float32
    f16 = mybir.dt.float16
    f8 = mybir.dt.float8e4
    DRM = mybir.MatmulPerfMode.DoubleRow

    xh8 = nc.dram_tensor("xh8", [D, ROWS], f8, kind="ExternalInput").ap()
    x16 = nc.dram_tensor("x16", [D, ROWS], f16, kind="ExternalInput").ap()
    wkvt = nc.dram_tensor("wkvt", [D, OD], f16, kind="ExternalInput").ap()
    wg8 = nc.dram_tensor("wg8", [D, OD], f8, kind="ExternalInput").ap()
    apeb = nc.dram_tensor("apeb", [128, OD], f32, kind="ExternalInput").ap()
    out = nc.dram_tensor("out", [ROWS // R * 2, HD], f32, kind="ExternalOutput").ap()

    # [p, dc, n] views with the contraction dim on partitions
    xh8_v = xh8.rearrange("(dc p) s -> p dc s", p=128)
    x16_v = x16.rearrange("(dc p) s -> p dc s", p=128)
    wkvt_v = wkvt.rearrange("(dc p) o -> p dc o", p=128)
    wg8_v = wg8.rearrange("(dc p) o -> p dc o", p=128)
    out_v = out.rearrange("(w two) h -> w two h", two=2)

    with tile.TileContext(nc) as tc:
        with (
            tc.tile_pool(name="const", bufs=1) as const_pool,
            tc.tile_pool(name="wpool", bufs=1) as wpool,
            tc.tile_pool(name="xpool", bufs=2) as xpool,
            tc.tile_pool(name="acts", bufs=2) as acts,
            tc.tile_pool(name="small", bufs=2) as small,
            tc.tile_pool(name="pp", bufs=NT) as p16_pool,
            tc.tile_pool(name="sq", bufs=NT * 2) as ssq_pool,
            tc.tile_pool(name="fin", bufs=4) as fin_pool,
            tc.tile_pool(name="mm", bufs=3, space="PSUM") as psum_pool,
        ):
            # Weight slices, one dc-pair per tile so a [:, :, cslice] view is
            # directly a DoubleRow rhs ([128, 2, 512]).
            wkv_sl, wg_sl = [], []
            xb0 = []
            XSL0 = 8  # dc chunks per block-0 x DMA slice

            for d in range(NP):
                if d % (NP // (DC // XSL0)) == 0:
                    s0 = d // (NP // (DC // XSL0))
                    t = const_pool.tile([128, XSL0, SBLK], f8,
                                        tag=f"x0_{s0}", name=f"x0_{s0}")
                    nc.sync.dma_start(
                        t[:], xh8_v[:, s0 * XSL0:(s0 + 1) * XSL0, 0:SBLK])
                    xb0.append(t)
                t = wpool.tile([128, 2, OD], f16, tag=f"wkv{d}")
                nc.sync.dma_start(t[:], wkvt_v[:, 2 * d:2 * d + 2, :])
                wkv_sl.append(t)
                t = wpool.tile([128, 2, OD], f8, tag=f"wg{d}")
                nc.sync.dma_start(t[:], wg8_v[:, 2 * d:2 * d + 2, :])
                wg_sl.append(t)

            apeb_sb = const_pool.tile([128, OD], f32)
            nc.sync.dma_start(apeb_sb[:], apeb)
            eps_sb = const_pool.tile([128, 1], f32)
            nc.gpsimd.memset(eps_sb[:], EPS)

            def load_xblk(key, view, dtype, blk):
                t = xpool.tile([128, DC, SBLK], dtype, tag=f"xt_{key}")
                nc.sync.dma_start(t[:], view[:, :, blk * SBLK:(blk + 1) * SBLK])
                return t

            def xop(xt, d, j):
                # lhsT [128, 2, 128] for DoubleRow: dc-pair d, s-tile j
                if isinstance(xt, list):  # block-0 dc-sliced tiles
                    t = xt[(2 * d) // XSL0]
                    return t[:, (2 * d) % XSL0:(2 * d) % XSL0 + 2,
                             j * 128:(j + 1) * 128]
                return xt[:, 2 * d:2 * d + 2, j * 128:(j + 1) * 128]

            def epilogue_mid(ps_kv, ps_g, i, c):
                # Per-group epilogue through the pooled sum-of-squares; the
                # sqrt-based finals are batched (epilogue_fin) so the ACT
                # table switches (sigmoid<->sqrt sets) are amortized.
                gate_sb = acts.tile([128, HD], f32, tag="gate")
                nc.scalar.activation(gate_sb[:], ps_g[:],
                                     mybir.ActivationFunctionType.Sigmoid,
                                     scale=1.0 / WSCALE)
                kvg_sb = acts.tile([128, HD], f32, tag="kvg")
                nc.vector.tensor_mul(kvg_sb[:], ps_kv[:], gate_sb[:])
                kvg16 = acts.tile([128, HD], f16, tag="kvg16")
                nc.vector.tensor_add(kvg16[:], kvg_sb[:],
                                     apeb_sb[:, c * HD:(c + 1) * HD])
                # Window-pool via accumulating DMA on the gpsimd queue:
                # wkv/ape are host-scaled x0.25 so this 4-row SUM equals the
                # reference mean. Frees the PE pool matmul + the ACT copy.
                p16 = p16_pool.tile([NW_TILE, HD], f16, tag="p16")
                for k in range(4):
                    nc.gpsimd.dma_start(
                        p16[:], kvg16[:][k:128:4, :],
                        accum_op=(mybir.AluOpType.bypass if k == 0
                                  else mybir.AluOpType.add))
                sq = small.tile([NW_TILE, HD], f32, tag="sqj")
                ssq = ssq_pool.tile([NW_TILE, 1], f32, tag="ssq")
                nc.scalar.activation(sq[:], p16[:],
                                     mybir.ActivationFunctionType.Square,
                                     accum_out=ssq[:])
                return (p16, ssq, i, c)

            def epilogue_fin(p16, ssq, i, c):
                std = small.tile([NW_TILE, 1], f32, tag="std")
                nc.scalar.activation(std[:], ssq[:],
                                     mybir.ActivationFunctionType.Sqrt,
                                     bias=eps_sb[:NW_TILE, :], scale=1.0 / HD)
                rinv = small.tile([NW_TILE, 1], f32, tag="rinv")
                nc.vector.reciprocal(rinv[:], std[:])
                onorm = fin_pool.tile([NW_TILE, HD], f32, tag="onorm")
                nc.scalar.mul(onorm[:], p16[:], rinv[:])
                nc.sync.dma_start(
                    out_v[i * NW_TILE:(i + 1) * NW_TILE, c, :], onorm[:])

            pending = []
            fins = []

            def flush(keep):
                while len(pending) > keep:
                    fins.append(epilogue_mid(*pending.pop(0)))

            def drain_fins():
                for f in fins:
                    epilogue_fin(*f)
                fins.clear()

            for _rep in range(reps):
                for blk in range(NT * 128 // SBLK):
                    first = _rep == 0 and blk == 0
                    kv_t = load_xblk("x16", x16_v, f16, blk)
                    xh_t = xb0 if first else load_xblk("xh", xh8_v, f8, blk)
                    for j in range(SBLK // 128):
                        for c in range(2):
                            i = blk * (SBLK // 128) + j
                            csl = slice(c * HD, (c + 1) * HD)
                            ps_kv = psum_pool.tile([128, HD], f32, tag="ps_kv")
                            ps_g = psum_pool.tile([128, HD], f32, tag="ps_g")
                            for dc in range(DC):
                                nc.tensor.matmul(
                                    ps_kv[:],
                                    kv_t[:, dc, j * 128:(j + 1) * 128],
                                    wkv_sl[dc // 2][:, dc % 2, csl],
                                    start=(dc == 0), stop=(dc == DC - 1))
                            for d in range(NP):
                                nc.tensor.matmul(
                                    ps_g[:], xop(xh_t, d, j),
                                    wg_sl[d][:, :, csl],
                                    start=(d == 0), stop=(d == NP - 1),
                                    perf_mode=DRM)
                            pending.append((ps_kv, ps_g, i, c))
                            flush(1)
                            if len(fins) >= NT:
                                drain_fins()
                flush(0)
                drain_fins()

    if do_compile:
        nc.compile()
    return nc


def _get_nc():
    global _CACHED_NC
    if _CACHED_NC is None:
        _CACHED_NC = _build_nc()
    return _CACHED_NC


def _prep_in_maps(x, wkv_w, wgate_w, ape, norm_w):
    x = np.asarray(x, dtype=np.float32)
    wkv_w = np.asarray(wkv_w, dtype=np.float32)
    wgate_w = np.asarray(wgate_w, dtype=np.float32)
    ape = np.asarray(ape, dtype=np.float32)

    xb = x.reshape(B * S, D)
    xh = xb.astype(F8)
    x16 = xb.astype(HALF)

    # wkv/ape carry the 1/R pooling factor so the on-device 4-row
    # accumulating-DMA SUM equals the reference window mean.
    common = {
        "wkvt": np.ascontiguousarray((wkv_w.T / R).astype(HALF)),    # [D, OD]
        "wg8": np.ascontiguousarray((wgate_w.T * WSCALE).astype(F8)),
        "apeb": np.ascontiguousarray(np.tile(ape / R, (128 // R, 1))),
    }

    in_maps = []
    for k in range(N_CORES):
        rows = slice(k * ROWS, (k + 1) * ROWS)
        m = dict(common)
        m["xh8"] = np.ascontiguousarray(xh[rows, :].T)    # [D, ROWS]
        m["x16"] = np.ascontiguousarray(x16[rows, :].T)
        in_maps.append(m)
    return in_maps


def kernel(x, wkv_w, wgate_w, ape, norm_w):
    nc = _get_nc()
    in_maps = _prep_in_maps(x, wkv_w, wgate_w, ape, norm_w)
    try:
        res = run_bass_kernel_spmd(nc, in_maps, list(range(N_CORES)))
    except Exception:
        # Transient axon-transport failures are retryable; a wedged device
        # (NRT_EXEC_UNIT_UNRECOVERABLE) recovers with a fresh PJRT session.
        try:
            import jax
            jax.clear_backends()
        except Exception:
            pass
        res = run_bass_kernel_spmd(nc, in_maps, list(range(N_CORES)))
    shards = [res.results[k]["out"] for k in range(N_CORES)]
    full = np.concatenate(shards, axis=0).reshape(B, S // R * 2, HD)
    # norm_w multiplies the output elementwise along the head dim; applied
    # on the host, exactly.
    return full * np.asarray(norm_w, dtype=np.float32)[None, None, :]
